# revision 31
# baseline (speedup 1.0000x reference)
"""Trainium2 Bass kernel for AVID similarity-with-positive-expansion.

For each batch element b and slot j in [y[b], pos_idx[b,:], neg_idx[b,:]]:
    v2a[b, j] = dot(view2_mem[idx], v_norm[b]) / T
    a2v[b, j] = dot(view1_mem[idx], a_norm[b]) / T

Strategy (data-parallel over batch across 8 NeuronCores, 32 batch
elements per core; fp16 memory banks replicated per core):
  - Host packs both banks fp16-interleaved: row r = [view2h[r] | view1h[r]]
    (512B), bins each core's 33056 row-indices into 8 windows of 32768
    rows (int16 local indices), padded to MAXN per window.
  - Device per window: ONE `dma_gather` (non-transpose, elem=256) pulls
    both modalities' rows; slot i lands at partition i%128, chunk i//128.
  - PE transposes each [128 slot, 128 d] chunk (identity matmul) into a
    packed PSUM tile [128 d, 512 slots] (fp16), DVE-copies it to SBUF,
    then one PE matmul scores ALL 32 queries against 512 slots.
  - Score chunks stack 4-deep into [128, 512] f32 PSUM tiles, are copied
    to SBUF and DMA'd out; host extracts row b(slot) per slot and
    reassembles (2, 256, 1033) fp32.
Queries are normalized + scaled by 1/T on device in fp32, transposed
(DVE 32x32 block transposes) and cast to fp16 for the PE.
"""

import sys

import numpy as np

sys.path.insert(0, "/opt/trn_rl_repo")

T = 0.07
EPS = 1e-12
MEM, DIM = 262144, 128
BS, NPOS, NNEG = 256, 8, 1024
NSLOT = 1 + NPOS + NNEG  # 1033
NCORES = 8
BSL = BS // NCORES  # 32
WIN = 32768  # rows per int16-indexable window
NWIN = MEM // WIN  # 8
MAXN_DEFAULT = 4608  # padded slots per window; multiple of 512
CHUNK = 512  # matmul N / PSUM bank

_CACHE = {}
_TRACE = False
LAST_EXEC_NS = None
LAST_RESULT = None


def _build_nc(maxn):
    from concourse import bacc, mybir, tile

    f32 = mybir.dt.float32
    f16 = mybir.dt.float16
    i16 = mybir.dt.int16
    mult = mybir.AluOpType.mult
    add = mybir.AluOpType.add

    nt = maxn // CHUNK  # rhs tiles (score matmuls) per window per mod
    ncg = NWIN * nt  # score chunks per mod
    assert ncg % 4 == 0
    nstage = ncg // 4  # output [128, 512] tiles per mod
    icols = maxn // 16
    nck = maxn // 128  # gathered chunks per window

    nc = bacc.Bacc(
        "TRN2", target_bir_lowering=False, debug=False,
        dynamic_dma_scratch_size=32768,
    )

    qv_ext = nc.dram_tensor("q_video", [BSL, DIM], f32, kind="ExternalInput")
    qa_ext = nc.dram_tensor("q_audio", [BSL, DIM], f32, kind="ExternalInput")
    vp_ext = nc.dram_tensor("viewp", [MEM, 2 * DIM], f16, kind="ExternalInput")
    idx_ext = nc.dram_tensor("idx16", [128, NWIN * icols], i16, kind="ExternalInput")
    id_ext = nc.dram_tensor("ident", [128, 128], f16, kind="ExternalInput")
    wc_ext = nc.dram_tensor(
        "wcnt", [1, 2 * NWIN], mybir.dt.int32, kind="ExternalInput"
    )
    out_ext = nc.dram_tensor(
        "scores", [2, nstage, 128, CHUNK], f32, kind="ExternalOutput"
    )

    qexts = [qv_ext, qa_ext]  # mod 0 (v2a) pairs with packed cols 0:128 (view2)

    with tile.TileContext(nc) as tc:
        with (
            tc.tile_pool(name="small", bufs=1) as small,
            tc.tile_pool(name="gpool", bufs=4) as gpool,
            tc.tile_pool(name="rpool", bufs=6) as rpool,
            tc.tile_pool(name="spool", bufs=4) as spool,
            tc.tile_pool(name="tpsum", bufs=3, space="PSUM") as tpsum,
            tc.tile_pool(name="spsum", bufs=2, space="PSUM") as spsum,
        ):
            idxt = small.tile([128, NWIN * icols], i16, tag="idxt")
            nc.sync.dma_start(idxt[:], idx_ext[:])
            wct = small.tile([1, 2 * NWIN], mybir.dt.int32, tag="wct")
            nc.sync.dma_start(wct[:], wc_ext[:])
            nreg = nc.gpsimd.alloc_register()

            ident = small.tile([128, 128], f16, tag="ident")
            nc.sync.dma_start(ident[:], id_ext[:])

            # ---- queries: normalize, scale 1/T, transpose, cast fp16
            qth = []
            for m in range(2):
                qt = small.tile([BSL, DIM], f32, tag=f"qt{m}")
                nc.sync.dma_start(qt[:], qexts[m][:])
                sq = small.tile([BSL, DIM], f32, tag=f"sq{m}")
                ss = small.tile([BSL, 1], f32, tag=f"ss{m}")
                nc.vector.tensor_tensor(out=sq[:], in0=qt[:], in1=qt[:], op=mult)
                nc.vector.tensor_reduce(
                    out=ss[:], in_=sq[:], axis=mybir.AxisListType.X, op=add
                )
                nrm = small.tile([BSL, 1], f32, tag=f"nrm{m}")
                nc.scalar.sqrt(nrm[:], ss[:])
                nrm2 = small.tile([BSL, 1], f32, tag=f"nrm2{m}")
                nc.vector.tensor_scalar_max(nrm2[:], nrm[:], EPS)
                rec = small.tile([BSL, 1], f32, tag=f"rec{m}")
                nc.vector.reciprocal(rec[:], nrm2[:])
                rec2 = small.tile([BSL, 1], f32, tag=f"rec2{m}")
                nc.scalar.mul(rec2[:], rec[:], 1.0 / T)
                qs = small.tile([BSL, DIM], f32, tag=f"qs{m}")
                nc.vector.tensor_scalar_mul(qs[:], qt[:], rec2[:])
                qT = small.tile([128, BSL], f32, tag=f"qT{m}")
                for j in range(4):
                    nc.vector.transpose(
                        out=qT[32 * j:32 * (j + 1), :],
                        in_=qs[:, 32 * j:32 * (j + 1)],
                    )
                qh = small.tile([128, BSL], f16, tag=f"qh{m}")
                nc.vector.tensor_copy(qh[:], qT[:])
                qth.append(qh)

            # ---- per window: gather pairs, transpose chunks, score
            # last window split into two half-gathers to shorten the tail
            cur_ps = [None, None]
            for w in range(NWIN):
                gt = gpool.tile([128, nck * 2 * DIM], f16, tag="g", name=f"g{w}")
                gt3 = gt[:].rearrange("p (c e) -> p c e", e=2 * DIM)
                segs = (
                    [(0, maxn // 2), (maxn // 2, maxn)] if w == NWIN - 1
                    else [(0, maxn)]
                )
                for si, (lo, hi) in enumerate(segs):
                    nc.gpsimd.reg_load(
                        nreg, wct[0:1, w + si * NWIN:w + si * NWIN + 1]
                    )
                    nc.gpsimd.dma_gather(
                        out_ap=gt3[:, lo // 128:hi // 128, :],
                        in_ap=vp_ext[w * WIN:(w + 1) * WIN, :],
                        idxs_ap=idxt[
                            :, w * icols + lo // 16:w * icols + hi // 16
                        ],
                        num_idxs=hi - lo,
                        num_idxs_reg=nreg,
                        elem_size=2 * DIM,
                        single_packet=False,
                    )
                for m in range(2):
                    for t in range(nt):
                        tp = tpsum.tile(
                            [128, CHUNK], f16, tag="tp", name=f"tp{w}_{m}_{t}"
                        )
                        for qc in range(4):
                            c = t * 4 + qc
                            nc.tensor.transpose(
                                out=tp[:, 128 * qc:128 * (qc + 1)],
                                in_=gt3[:, c, 128 * m:128 * (m + 1)],
                                identity=ident[:],
                            )
                        rhs = rpool.tile(
                            [128, CHUNK], f16, tag="rhs", name=f"rhs{w}_{m}_{t}"
                        )
                        nc.vector.tensor_copy(rhs[:], tp[:])
                        cg = w * nt + t
                        quad = cg % 4
                        if quad == 0:
                            cur_ps[m] = spsum.tile(
                                [128, CHUNK], f32, tag=f"ps{m}", name=f"ps{m}_{cg}"
                            )
                        nc.tensor.matmul(
                            out=cur_ps[m][32 * quad:32 * (quad + 1), :],
                            lhsT=qth[m][:],
                            rhs=rhs[:],
                            tile_position=(0, 32 * quad),
                            start=True,
                            stop=True,
                        )
                        if quad == 3:
                            stg = spool.tile(
                                [128, CHUNK], f32, tag=f"st{m}", name=f"st{m}_{cg}"
                            )
                            nc.vector.tensor_copy(stg[:], cur_ps[m][:])
                            nc.sync.dma_start(out_ext[m, cg // 4], stg[:])

    nc.compile()
    return nc


def _get_nc(maxn):
    if maxn not in _CACHE:
        _CACHE[maxn] = _build_nc(maxn)
    return _CACHE[maxn]


def _round_up(x, m):
    return (x + m - 1) // m * m


def _prep(video_emb, audio_emb, view1_mem, view2_mem, y, pos_idx, neg_idx):
    """Build per-core input maps + slot bookkeeping. Returns
    (maxn, in_maps, books) where books[r][w] = (positions, b, j)."""
    v = np.ascontiguousarray(np.asarray(video_emb, dtype=np.float32))
    a = np.ascontiguousarray(np.asarray(audio_emb, dtype=np.float32))
    m1h = np.asarray(view1_mem, dtype=np.float32).astype(np.float16)
    m2h = np.asarray(view2_mem, dtype=np.float32).astype(np.float16)
    viewp = np.ascontiguousarray(np.concatenate([m2h, m1h], axis=1))
    yi = np.asarray(y).astype(np.int64)
    pi = np.asarray(pos_idx).astype(np.int64)
    ni = np.asarray(neg_idx).astype(np.int64)

    all_idx = np.concatenate([yi[:, None], pi, ni], axis=1)  # (256, 1033)
    per_core = []
    maxcount = 0
    for r in range(NCORES):
        ci = all_idx[r * BSL:(r + 1) * BSL].reshape(-1)  # slot s = b*1033 + j
        wins = []
        for w in range(NWIN):
            posn = np.nonzero((ci >= w * WIN) & (ci < (w + 1) * WIN))[0]
            loc = ci[posn] - w * WIN
            uniq, inv = np.unique(loc, return_inverse=True)
            if len(uniq) == 0:
                uniq = np.zeros(1, np.int64)
            wins.append((posn, uniq.astype(np.int16), inv))
            maxcount = max(maxcount, len(uniq))
        per_core.append(wins)

    maxn = max(MAXN_DEFAULT, _round_up(maxcount, 2048))
    icols = maxn // 16

    in_maps, books = [], []
    for r in range(NCORES):
        idx16 = np.full((128, NWIN * icols), -1, np.int16)
        wcnt = np.zeros((1, 2 * NWIN), np.int32)
        book = []
        half = maxn // 2
        for w in range(NWIN):
            posn, uniq, inv = per_core[r][w]
            n = len(uniq)
            t = np.full((16, icols), -1, np.int16)
            i = np.arange(n)
            t[i % 16, i // 16] = uniq
            if w == NWIN - 1:
                # split window: per-segment counts, each segment >= 1 valid
                wcnt[0, w] = min(n, half)
                n2 = n - half
                if n2 <= 0:
                    t[0, half // 16] = 0  # dummy valid idx for empty segment
                    n2 = 1
                wcnt[0, NWIN + w] = n2
            else:
                wcnt[0, w] = n
            idx16[:, w * icols:(w + 1) * icols] = np.tile(t, (8, 1))
            book.append((posn, posn // NSLOT, posn % NSLOT, inv))
        in_maps.append({
            "q_video": v[r * BSL:(r + 1) * BSL],
            "q_audio": a[r * BSL:(r + 1) * BSL],
            "viewp": viewp,
            "idx16": idx16,
            "ident": np.eye(128, dtype=np.float16),
            "wcnt": wcnt,
        })
        books.append(book)
    return maxn, in_maps, books


def _decode(results, books, maxn):
    nt = maxn // CHUNK
    out = np.empty((2, BS, NSLOT), np.float32)
    for r in range(NCORES):
        sc = np.asarray(results[r]["scores"])  # [2, nstage, 128, 512]
        for m in range(2):
            for w in range(NWIN):
                posn, b, j, inv = books[r][w]
                p = inv % 128
                c = inv // 128
                t = c // 4
                col = (c % 4) * 128 + p
                cg = w * nt + t
                out[m, r * BSL + b, j] = sc[m, cg // 4, 32 * (cg % 4) + b, col]
    return out


def kernel(video_emb, audio_emb, view1_mem, view2_mem, y, pos_idx, neg_idx):
    global LAST_EXEC_NS, LAST_RESULT
    from concourse.bass_utils import run_bass_kernel_spmd

    maxn, in_maps, books = _prep(
        video_emb, audio_emb, view1_mem, view2_mem, y, pos_idx, neg_idx
    )
    nc = _get_nc(maxn)
    res = run_bass_kernel_spmd(
        nc, in_maps, core_ids=list(range(NCORES)), trace=_TRACE
    )
    LAST_EXEC_NS = res.exec_time_ns
    LAST_RESULT = res
    return _decode(res.results, books, maxn)


def simulate_core0(video_emb, audio_emb, view1_mem, view2_mem, y, pos_idx, neg_idx):
    """CoreSim (host simulator) of core 0 only; returns (2, 32, 1033)."""
    from concourse.bass_interp import CoreSim

    maxn, in_maps, books = _prep(
        video_emb, audio_emb, view1_mem, view2_mem, y, pos_idx, neg_idx
    )
    nc = _get_nc(maxn)
    sim = CoreSim(nc, require_finite=False, require_nnan=False)
    for k, val in in_maps[0].items():
        sim.tensor(k)[:] = val
    sim.simulate()
    nstage = NWIN * (maxn // CHUNK) // 4
    results = [{"scores": np.array(sim.tensor("scores"))}] + [
        {"scores": np.zeros((2, nstage, 128, CHUNK), np.float32)}
    ] * (NCORES - 1)
    return _decode(results, books, maxn)[:, :BSL]


# revision 33
# speedup vs baseline: 1.1694x; 1.1694x over previous
"""Trainium2 Bass kernel for AVID similarity-with-positive-expansion.

For each batch element b and slot j in [y[b], pos_idx[b,:], neg_idx[b,:]]:
    v2a[b, j] = dot(view2_mem[idx], v_norm[b]) / T
    a2v[b, j] = dot(view1_mem[idx], a_norm[b]) / T

Strategy (data-parallel over batch across 8 NeuronCores, 32 batch
elements per core; fp16 memory banks replicated per core):
  - Host packs both banks fp16-interleaved: row r = [view2h[r] | view1h[r]]
    (512B), bins each core's 33056 row-indices into 8 windows of 32768
    rows (int16 local indices), padded to MAXN per window.
  - Device per window: ONE `dma_gather` (non-transpose, elem=256) pulls
    both modalities' rows; slot i lands at partition i%128, chunk i//128.
  - PE transposes each [128 slot, 128 d] chunk (identity matmul) into a
    packed PSUM tile [128 d, 512 slots] (fp16), DVE-copies it to SBUF,
    then one PE matmul scores ALL 32 queries against 512 slots.
  - Score chunks stack 4-deep into [128, 512] f32 PSUM tiles, are copied
    to SBUF and DMA'd out; host extracts row b(slot) per slot and
    reassembles (2, 256, 1033) fp32.
Queries are normalized + scaled by 1/T on device in fp32, transposed
(DVE 32x32 block transposes) and cast to fp16 for the PE.
"""

import sys

import numpy as np

sys.path.insert(0, "/opt/trn_rl_repo")

T = 0.07
EPS = 1e-12
MEM, DIM = 262144, 128
BS, NPOS, NNEG = 256, 8, 1024
NSLOT = 1 + NPOS + NNEG  # 1033
NCORES = 8
BSL = BS // NCORES  # 32
WIN = 32768  # rows per int16-indexable window
NWIN = MEM // WIN  # 8
MAXN_DEFAULT = 4608  # padded slots per window; multiple of 512
CHUNK = 512  # matmul N / PSUM bank

_CACHE = {}
_TRACE = False
LAST_EXEC_NS = None
LAST_RESULT = None


def _build_nc(maxn):
    from concourse import bacc, mybir, tile

    f32 = mybir.dt.float32
    f16 = mybir.dt.float16
    i16 = mybir.dt.int16
    mult = mybir.AluOpType.mult
    add = mybir.AluOpType.add

    nt = maxn // CHUNK  # rhs tiles (score matmuls) per window per mod
    ncg = NWIN * nt  # score chunks per mod
    assert ncg % 4 == 0
    nstage = ncg // 4  # output [128, 512] tiles per mod
    icols = maxn // 16
    nck = maxn // 128  # gathered chunks per window

    nc = bacc.Bacc(
        "TRN2", target_bir_lowering=False, debug=False,
        dynamic_dma_scratch_size=32768,
    )

    qv_ext = nc.dram_tensor("q_video", [BSL, DIM], f32, kind="ExternalInput")
    qa_ext = nc.dram_tensor("q_audio", [BSL, DIM], f32, kind="ExternalInput")
    vp_ext = nc.dram_tensor("viewp", [MEM, 2 * DIM], f16, kind="ExternalInput")
    idx_ext = nc.dram_tensor("idx16", [128, NWIN * icols], i16, kind="ExternalInput")
    id_ext = nc.dram_tensor("ident", [128, 128], f16, kind="ExternalInput")
    wc_ext = nc.dram_tensor(
        "wcnt", [1, 2 * NWIN], mybir.dt.int32, kind="ExternalInput"
    )
    out_ext = nc.dram_tensor(
        "scores", [2, nstage, 128, CHUNK], f32, kind="ExternalOutput"
    )

    qexts = [qv_ext, qa_ext]  # mod 0 (v2a) pairs with packed cols 0:128 (view2)

    with tile.TileContext(nc) as tc:
        with (
            tc.tile_pool(name="small", bufs=1) as small,
            tc.tile_pool(name="gpool", bufs=4) as gpool,
            tc.tile_pool(name="rpool", bufs=6) as rpool,
            tc.tile_pool(name="spool", bufs=4) as spool,
            tc.tile_pool(name="tpsum", bufs=3, space="PSUM") as tpsum,
            tc.tile_pool(name="spsum", bufs=2, space="PSUM") as spsum,
        ):
            idxt = small.tile([128, NWIN * icols], i16, tag="idxt")
            nc.sync.dma_start(idxt[:], idx_ext[:])
            wct = small.tile([1, 2 * NWIN], mybir.dt.int32, tag="wct")
            nc.sync.dma_start(wct[:], wc_ext[:])
            nreg = nc.gpsimd.alloc_register()

            ident = small.tile([128, 128], f16, tag="ident")
            nc.sync.dma_start(ident[:], id_ext[:])

            # ---- queries: normalize, scale 1/T, transpose, cast fp16
            qth = []
            for m in range(2):
                qt = small.tile([BSL, DIM], f32, tag=f"qt{m}")
                nc.sync.dma_start(qt[:], qexts[m][:])
                sq = small.tile([BSL, DIM], f32, tag=f"sq{m}")
                ss = small.tile([BSL, 1], f32, tag=f"ss{m}")
                nc.vector.tensor_tensor(out=sq[:], in0=qt[:], in1=qt[:], op=mult)
                nc.vector.tensor_reduce(
                    out=ss[:], in_=sq[:], axis=mybir.AxisListType.X, op=add
                )
                nrm = small.tile([BSL, 1], f32, tag=f"nrm{m}")
                nc.scalar.sqrt(nrm[:], ss[:])
                nrm2 = small.tile([BSL, 1], f32, tag=f"nrm2{m}")
                nc.vector.tensor_scalar_max(nrm2[:], nrm[:], EPS)
                rec = small.tile([BSL, 1], f32, tag=f"rec{m}")
                nc.vector.reciprocal(rec[:], nrm2[:])
                rec2 = small.tile([BSL, 1], f32, tag=f"rec2{m}")
                nc.scalar.mul(rec2[:], rec[:], 1.0 / T)
                qs = small.tile([BSL, DIM], f32, tag=f"qs{m}")
                nc.vector.tensor_scalar_mul(qs[:], qt[:], rec2[:])
                qT = small.tile([128, BSL], f32, tag=f"qT{m}")
                for j in range(4):
                    nc.vector.transpose(
                        out=qT[32 * j:32 * (j + 1), :],
                        in_=qs[:, 32 * j:32 * (j + 1)],
                    )
                qh = small.tile([128, BSL], f16, tag=f"qh{m}")
                nc.vector.tensor_copy(qh[:], qT[:])
                qth.append(qh)

            # ---- per window: gather pairs, transpose chunks, score
            # last window split into two half-gathers to shorten the tail
            cur_ps = [None, None]
            for w in range(NWIN):
                gt = gpool.tile([128, nck * 2 * DIM], f16, tag="g", name=f"g{w}")
                gt3 = gt[:].rearrange("p (c e) -> p c e", e=2 * DIM)
                segs = [(0, maxn)]
                for si, (lo, hi) in enumerate(segs):
                    nc.gpsimd.reg_load(
                        nreg, wct[0:1, w + si * NWIN:w + si * NWIN + 1]
                    )
                    nc.gpsimd.dma_gather(
                        out_ap=gt3[:, lo // 128:hi // 128, :],
                        in_ap=vp_ext[w * WIN:(w + 1) * WIN, :],
                        idxs_ap=idxt[
                            :, w * icols + lo // 16:w * icols + hi // 16
                        ],
                        num_idxs=hi - lo,
                        num_idxs_reg=nreg,
                        elem_size=2 * DIM,
                        single_packet=False,
                    )
                for m in range(2):
                    for t in range(nt):
                        tp = tpsum.tile(
                            [128, CHUNK], f16, tag="tp", name=f"tp{w}_{m}_{t}"
                        )
                        for qc in range(4):
                            c = t * 4 + qc
                            nc.tensor.transpose(
                                out=tp[:, 128 * qc:128 * (qc + 1)],
                                in_=gt3[:, c, 128 * m:128 * (m + 1)],
                                identity=ident[:],
                            )
                        rhs = rpool.tile(
                            [128, CHUNK], f16, tag="rhs", name=f"rhs{w}_{m}_{t}"
                        )
                        nc.vector.tensor_copy(rhs[:], tp[:])
                        cg = w * nt + t
                        quad = cg % 4
                        if quad == 0:
                            cur_ps[m] = spsum.tile(
                                [128, CHUNK], f32, tag=f"ps{m}", name=f"ps{m}_{cg}"
                            )
                        nc.tensor.matmul(
                            out=cur_ps[m][32 * quad:32 * (quad + 1), :],
                            lhsT=qth[m][:],
                            rhs=rhs[:],
                            tile_position=(0, 32 * quad),
                            start=True,
                            stop=True,
                        )
                        if quad == 3:
                            stg = spool.tile(
                                [128, CHUNK], f32, tag=f"st{m}", name=f"st{m}_{cg}"
                            )
                            nc.vector.tensor_copy(stg[:], cur_ps[m][:])
                            nc.sync.dma_start(out_ext[m, cg // 4], stg[:])

    nc.compile()
    return nc


def _get_nc(maxn):
    if maxn not in _CACHE:
        _CACHE[maxn] = _build_nc(maxn)
    return _CACHE[maxn]


def _round_up(x, m):
    return (x + m - 1) // m * m


def _prep(video_emb, audio_emb, view1_mem, view2_mem, y, pos_idx, neg_idx):
    """Build per-core input maps + slot bookkeeping. Returns
    (maxn, in_maps, books) where books[r][w] = (positions, b, j)."""
    v = np.ascontiguousarray(np.asarray(video_emb, dtype=np.float32))
    a = np.ascontiguousarray(np.asarray(audio_emb, dtype=np.float32))
    m1h = np.asarray(view1_mem, dtype=np.float32).astype(np.float16)
    m2h = np.asarray(view2_mem, dtype=np.float32).astype(np.float16)
    viewp = np.ascontiguousarray(np.concatenate([m2h, m1h], axis=1))
    yi = np.asarray(y).astype(np.int64)
    pi = np.asarray(pos_idx).astype(np.int64)
    ni = np.asarray(neg_idx).astype(np.int64)

    all_idx = np.concatenate([yi[:, None], pi, ni], axis=1)  # (256, 1033)
    per_core = []
    maxcount = 0
    for r in range(NCORES):
        ci = all_idx[r * BSL:(r + 1) * BSL].reshape(-1)  # slot s = b*1033 + j
        wins = []
        for w in range(NWIN):
            posn = np.nonzero((ci >= w * WIN) & (ci < (w + 1) * WIN))[0]
            loc = ci[posn] - w * WIN
            uniq, inv = np.unique(loc, return_inverse=True)
            if len(uniq) == 0:
                uniq = np.zeros(1, np.int64)
            wins.append((posn, uniq.astype(np.int16), inv))
            maxcount = max(maxcount, len(uniq))
        per_core.append(wins)

    maxn = max(MAXN_DEFAULT, _round_up(maxcount, 2048))
    icols = maxn // 16

    in_maps, books = [], []
    for r in range(NCORES):
        idx16 = np.full((128, NWIN * icols), -1, np.int16)
        wcnt = np.zeros((1, 2 * NWIN), np.int32)
        book = []
        half = maxn // 2
        for w in range(NWIN):
            posn, uniq, inv = per_core[r][w]
            n = len(uniq)
            t = np.full((16, icols), -1, np.int16)
            i = np.arange(n)
            t[i % 16, i // 16] = uniq
            wcnt[0, w] = n
            idx16[:, w * icols:(w + 1) * icols] = np.tile(t, (8, 1))
            book.append((posn, posn // NSLOT, posn % NSLOT, inv))
        in_maps.append({
            "q_video": v[r * BSL:(r + 1) * BSL],
            "q_audio": a[r * BSL:(r + 1) * BSL],
            "viewp": viewp,
            "idx16": idx16,
            "ident": np.eye(128, dtype=np.float16),
            "wcnt": wcnt,
        })
        books.append(book)
    return maxn, in_maps, books


def _decode(results, books, maxn):
    nt = maxn // CHUNK
    out = np.empty((2, BS, NSLOT), np.float32)
    for r in range(NCORES):
        sc = np.asarray(results[r]["scores"])  # [2, nstage, 128, 512]
        for m in range(2):
            for w in range(NWIN):
                posn, b, j, inv = books[r][w]
                p = inv % 128
                c = inv // 128
                t = c // 4
                col = (c % 4) * 128 + p
                cg = w * nt + t
                out[m, r * BSL + b, j] = sc[m, cg // 4, 32 * (cg % 4) + b, col]
    return out


def kernel(video_emb, audio_emb, view1_mem, view2_mem, y, pos_idx, neg_idx):
    global LAST_EXEC_NS, LAST_RESULT
    from concourse.bass_utils import run_bass_kernel_spmd

    maxn, in_maps, books = _prep(
        video_emb, audio_emb, view1_mem, view2_mem, y, pos_idx, neg_idx
    )
    nc = _get_nc(maxn)
    res = run_bass_kernel_spmd(
        nc, in_maps, core_ids=list(range(NCORES)), trace=_TRACE
    )
    LAST_EXEC_NS = res.exec_time_ns
    LAST_RESULT = res
    return _decode(res.results, books, maxn)


def simulate_core0(video_emb, audio_emb, view1_mem, view2_mem, y, pos_idx, neg_idx):
    """CoreSim (host simulator) of core 0 only; returns (2, 32, 1033)."""
    from concourse.bass_interp import CoreSim

    maxn, in_maps, books = _prep(
        video_emb, audio_emb, view1_mem, view2_mem, y, pos_idx, neg_idx
    )
    nc = _get_nc(maxn)
    sim = CoreSim(nc, require_finite=False, require_nnan=False)
    for k, val in in_maps[0].items():
        sim.tensor(k)[:] = val
    sim.simulate()
    nstage = NWIN * (maxn // CHUNK) // 4
    results = [{"scores": np.array(sim.tensor("scores"))}] + [
        {"scores": np.zeros((2, nstage, 128, CHUNK), np.float32)}
    ] * (NCORES - 1)
    return _decode(results, books, maxn)[:, :BSL]
